# revision 2
# baseline (speedup 1.0000x reference)
"""Causal self-attention (B=4, T=2048, D=1024, H=16) on 8 NeuronCores.

Sharding: core c handles batch b = c//2 and head-group hg = c%2 (8 heads each).
Per core: QKV projection (q,k in transposed [d,T] layout, v natural [T,d]),
causal flash-style attention in S^T layout (keys on partitions), partial
out-projection. Host sums the two partial outputs per batch element
(the "all-reduce" of the tensor-parallel out-projection).

All matmuls run in float32r (full PE rate at N>=256, ~1.6e-4 rel precision).
Softmax skips the max-subtraction (logits are O(5); exp is safe in fp32) and
the row-sum is produced for free by appending a ones-column to V. Per-query
normalization happens via reciprocal + PE rank-1 broadcast.
"""
import numpy as np

import concourse.bass as bass
import concourse.mybir as mybir
from concourse import bacc
from concourse.tile import TileContext
from concourse.bass_utils import run_bass_kernel_spmd

F32 = mybir.dt.float32
F32R = mybir.dt.float32r
Exp = mybir.ActivationFunctionType.Exp
Alu = mybir.AluOpType

B, T, D, H, HD = 4, 2048, 1024, 16, 64
NCORES = 8
TB = 512                  # q-block / N-block size
NTB = T // TB             # 4 q-blocks
NT = T // 128             # 16 t-tiles
NKD = D // 128            # 8 contraction tiles for the projections
NHP = 4                   # head-pairs per core (8 heads)
NEG = -1.0e30


def build_nc():
    nc = bacc.Bacc("TRN2", target_bir_lowering=False, debug=False, num_devices=NCORES)
    xT = nc.declare_dram_parameter("xT", [D, T], F32R, isOutput=False)
    wq = nc.declare_dram_parameter("wq", [D, 512], F32R, isOutput=False)
    wk = nc.declare_dram_parameter("wk", [D, 512], F32R, isOutput=False)
    wv = nc.declare_dram_parameter("wv", [D, 512], F32R, isOutput=False)
    wo = nc.declare_dram_parameter("wo", [512, D], F32R, isOutput=False)
    msk = nc.declare_dram_parameter("msk", [4, 128, TB], F32, isOutput=False)
    ones = nc.declare_dram_parameter("ones", [128, 64], F32R, isOutput=False)
    out = nc.declare_dram_parameter("out", [T, D], F32, isOutput=True)

    with TileContext(nc) as tc:
        with (
            tc.tile_pool(name="sb", bufs=1) as sb,
            tc.tile_pool(name="ps", bufs=1, space="PSUM") as ps,
        ):
            # ---------- constants ----------
            ones_sb = sb.tile([128, 64], F32R, name="ones", tag="ones", bufs=1)
            nc.sync.dma_start(out=ones_sb, in_=ones[:, :])
            msk_sb = []
            for m in range(4):
                mt = sb.tile([128, TB], F32, name=f"msk{m}", tag=f"msk{m}", bufs=1)
                nc.sync.dma_start(out=mt, in_=msk[m])
                msk_sb.append(mt)

            # ---------- resident weights ----------
            # wq/wk as lhsT tiles [d-tile 128, 512 cols]; wv as rhs tiles.
            wq_sb, wk_sb, wv_sb = [], [], []
            for kd in range(NKD):
                tq = sb.tile([128, 512], F32R, name=f"wq{kd}", tag="wy", bufs=25)
                nc.sync.dma_start(out=tq, in_=wq[kd * 128:(kd + 1) * 128, :])
                wq_sb.append(tq)
                tk = sb.tile([128, 512], F32R, name=f"wk{kd}", tag="wy", bufs=25)
                nc.sync.dma_start(out=tk, in_=wk[kd * 128:(kd + 1) * 128, :])
                wk_sb.append(tk)
                tv = sb.tile([128, 512], F32R, name=f"wv{kd}", tag="wy", bufs=25)
                nc.sync.dma_start(out=tv, in_=wv[kd * 128:(kd + 1) * 128, :])
                wv_sb.append(tv)
            wo_sb = []
            for r in range(4):
                to = sb.tile([128, D], F32R, name=f"wo{r}", tag="wo", bufs=4)
                nc.sync.dma_start(out=to, in_=wo[r * 128:(r + 1) * 128, :])
                wo_sb.append(to)

            # ---------- phase A: qkv projections ----------
            # qT/kT: [128 (head-pair dims), T] tiles, one per head-pair, f32r
            qT_sb = [sb.tile([128, T], F32R, name=f"qT{hp}", tag="qkt", bufs=8)
                     for hp in range(NHP)]
            kT_sb = [sb.tile([128, T], F32R, name=f"kT{hp}", tag="qkt", bufs=8)
                     for hp in range(NHP)]
            # v natural + ones column: per t-tile [128, 8*65]
            v_sb = [sb.tile([128, 8 * 65], F32R, name=f"v{ti}", tag="v", bufs=16)
                    for ti in range(NT)]

            for tb in range(NTB):
                xt = []
                for kd in range(NKD):
                    t = sb.tile([128, TB], F32R, name=f"xt{tb}_{kd}", tag="xp", bufs=10)
                    nc.sync.dma_start(
                        out=t, in_=xT[kd * 128:(kd + 1) * 128, tb * TB:(tb + 1) * TB])
                    xt.append(t)
                # q/k transposed projections: psum [w-cols 128, t 512]
                for sec, (w_tiles, dst) in enumerate(((wq_sb, qT_sb), (wk_sb, kT_sb))):
                    for mc in range(4):
                        pacc = ps.tile([128, TB], F32, name=f"pqk{tb}_{sec}_{mc}",
                                       tag="mm", bufs=2)
                        for kd in range(NKD):
                            nc.tensor.matmul(
                                pacc, lhsT=w_tiles[kd][:, mc * 128:(mc + 1) * 128],
                                rhs=xt[kd], start=(kd == 0), stop=(kd == NKD - 1))
                        nc.vector.tensor_copy(
                            dst[mc][:, tb * TB:(tb + 1) * TB], pacc)
                # v natural projection: psum [t 128, 512 cols]
                for tt in range(4):
                    ti = tb * 4 + tt
                    pv = ps.tile([128, 512], F32, name=f"pv{ti}", tag="mm", bufs=2)
                    for kd in range(NKD):
                        nc.tensor.matmul(
                            pv, lhsT=xt[kd][:, tt * 128:(tt + 1) * 128],
                            rhs=wv_sb[kd], start=(kd == 0), stop=(kd == NKD - 1))
                    v3 = v_sb[ti].rearrange("p (h c) -> p h c", h=8)
                    nc.vector.tensor_copy(
                        v3[:, :, 0:64], pv.rearrange("p (h c) -> p h c", h=8))
                    nc.vector.tensor_copy(
                        v3[:, :, 64:65],
                        ones_sb[:, 0:8].rearrange("p (h c) -> p h c", h=8))

            # ---------- phase B: attention ----------
            yT_sb = {}
            for hp in range(NHP):
                for qb in range(NTB):
                    jmax = 4 * qb + 4
                    ypair = [ps.tile([65, TB], F32, name=f"y{hp}_{qb}_{i}",
                                     tag="y", bufs=2) for i in range(2)]
                    for j in range(jmax):
                        stp = [ps.tile([128, TB], F32, name=f"st{hp}_{qb}_{j}_{i}",
                                       tag="st", bufs=4) for i in range(2)]
                        for i in range(2):
                            nc.tensor.matmul(
                                stp[i],
                                lhsT=kT_sb[hp][i * 64:(i + 1) * 64, j * 128:(j + 1) * 128],
                                rhs=qT_sb[hp][i * 64:(i + 1) * 64, qb * TB:(qb + 1) * TB],
                                start=True, stop=True, tile_position=(i * 64, 0))
                        m = j - 4 * qb
                        for i in range(2):
                            if m >= 0:
                                nc.vector.tensor_tensor(
                                    out=stp[i], in0=stp[i], in1=msk_sb[m], op=Alu.add)
                            pt = sb.tile([128, TB], F32R, name=f"pt{hp}_{qb}_{j}_{i}",
                                         tag="xp", bufs=10)
                            nc.scalar.activation(pt, stp[i], Exp)
                            nc.tensor.matmul(
                                ypair[i], lhsT=v_sb[j][:, (2 * hp + i) * 65:(2 * hp + i + 1) * 65],
                                rhs=pt, start=(j == 0), stop=(j == jmax - 1))
                    # normalization: yT = y / sums
                    for i in range(2):
                        rc = sb.tile([1, TB], F32R, name=f"rc{hp}_{qb}_{i}",
                                     tag="rc", bufs=2)
                        with nc.allow_low_precision(reason="softmax denom reciprocal"):
                            nc.vector.reciprocal(rc, ypair[i][64:65, :])
                        bps = ps.tile([64, TB], F32, name=f"b{hp}_{qb}_{i}",
                                      tag="mm", bufs=2)
                        nc.tensor.matmul(bps, lhsT=ones_sb[0:1, :],
                                         rhs=rc, start=True, stop=True)
                        bc = sb.tile([64, TB], F32, name=f"bc{hp}_{qb}_{i}",
                                     tag="bc", bufs=3)
                        nc.scalar.copy(bc, bps)
                        yt = sb.tile([128, TB], F32R, name=f"yt{hp}_{qb}",
                                     tag="wy", bufs=25) if i == 0 else yT_sb[(hp, qb)]
                        yT_sb[(hp, qb)] = yt
                        nc.vector.tensor_tensor(
                            out=yt[i * 64:(i + 1) * 64, :],
                            in0=ypair[i][0:64, :], in1=bc, op=Alu.mult)

            # ---------- phase C: out-projection (partial; host all-reduces) ----------
            for ti in range(NT):
                qb, tt = divmod(ti, 4)
                po = [ps.tile([128, 512], F32, name=f"po{ti}_{e}", tag="mm", bufs=2)
                      for e in range(2)]
                for r in range(4):
                    lhsT = yT_sb[(r, qb)][:, tt * 128:(tt + 1) * 128]
                    for e in range(2):
                        nc.tensor.matmul(po[e], lhsT=lhsT,
                                         rhs=wo_sb[r][:, e * 512:(e + 1) * 512],
                                         start=(r == 0), stop=(r == 3))
                for e in range(2):
                    stg = sb.tile([128, 512], F32, name=f"stg{ti}_{e}",
                                  tag="stg", bufs=3)
                    nc.vector.tensor_copy(stg, po[e])
                    nc.sync.dma_start(
                        out=out[ti * 128:(ti + 1) * 128, e * 512:(e + 1) * 512],
                        in_=stg)
    nc.compile()
    return nc


def make_in_maps(x, w_qkv, w_out):
    x = np.asarray(x, np.float32)
    w_qkv = np.asarray(w_qkv, np.float32)
    w_out = np.asarray(w_out, np.float32)
    # causal mask patterns for the 4 diagonal j-tile offsets (ST layout: j on
    # partitions, q on free dim): allowed iff m*128 + p <= f
    m_idx = np.arange(4)[:, None, None] * 128 + np.arange(128)[None, :, None]
    f_idx = np.arange(TB)[None, None, :]
    msk = np.where(m_idx <= f_idx, 0.0, NEG).astype(np.float32)
    ones = np.ones((128, 64), np.float32)
    in_maps = []
    for c in range(NCORES):
        b, hg = divmod(c, 2)
        cs = slice(hg * 512, (hg + 1) * 512)
        in_maps.append({
            "xT": np.ascontiguousarray(x[b].T),
            "wq": np.ascontiguousarray(w_qkv[:, 0:D][:, cs] * 0.125),
            "wk": np.ascontiguousarray(w_qkv[:, D:2 * D][:, cs]),
            "wv": np.ascontiguousarray(w_qkv[:, 2 * D:3 * D][:, cs]),
            "wo": np.ascontiguousarray(w_out[cs, :]),
            "msk": msk,
            "ones": ones,
        })
    return in_maps


_NC_CACHE = []


def kernel(x, w_qkv, w_out):
    if not _NC_CACHE:
        _NC_CACHE.append(build_nc())
    nc = _NC_CACHE[0]
    in_maps = make_in_maps(x, w_qkv, w_out)
    res = run_bass_kernel_spmd(nc, in_maps, list(range(NCORES))).results
    out = np.empty((B, T, D), np.float32)
    for b in range(B):
        out[b] = res[2 * b]["out"] + res[2 * b + 1]["out"]
    return out


if __name__ == "__main__":
    rng = np.random.default_rng(0)
    x = rng.standard_normal((B, T, D)).astype(np.float32)
    w_qkv = (rng.standard_normal((D, 3 * D)) / np.sqrt(D)).astype(np.float32)
    w_out = (rng.standard_normal((D, D)) / np.sqrt(D)).astype(np.float32)
    y = kernel(x, w_qkv, w_out)
    print("ran ok", y.shape, y.dtype)


# revision 21
# speedup vs baseline: 1.2129x; 1.2129x over previous
"""Causal self-attention (B=4, T=2048, D=1024, H=16) on 8 NeuronCores.

Sharding: core c handles batch b = c//2 and head-group hg = c%2 (8 heads each).
Per core: QKV projection (q,k in transposed [d,T] layout, v natural [T,d]),
causal flash-style attention in S^T layout (keys on partitions), partial
out-projection. Host sums the two partial outputs per batch element
(the "all-reduce" of the tensor-parallel out-projection).

All matmuls run in float32r (full PE rate at N>=256, ~1.6e-4 rel precision).
Softmax skips the max-subtraction (logits are O(5); exp is safe in fp32) and
the row-sum is produced for free by appending a ones-column to V. Per-query
normalization happens via reciprocal + PE rank-1 broadcast. Diagonal blocks
are triangularly trimmed: mask/exp/AV only touch valid q-columns.
"""
import numpy as np

import concourse.bass as bass
import concourse.mybir as mybir
from concourse import bacc
from concourse.tile import TileContext
from concourse.bass_utils import run_bass_kernel_spmd

F32 = mybir.dt.float32
F32R = mybir.dt.float32r
Exp = mybir.ActivationFunctionType.Exp
Alu = mybir.AluOpType

B, T, D, H, HD = 4, 2048, 1024, 16, 64
NCORES = 8
TB = 512                  # q-block / N-block size
NTB = T // TB             # 4 q-blocks
NT = T // 128             # 16 t-tiles
NKD = D // 128            # 8 contraction tiles for the projections
NHP = 4                   # head-pairs per core (8 heads)
NEG = -1.0e30

# tuning knobs (SBUF budget guard: keep total <= ~207KB/partition)
CFG = dict(FUSE=1, XP=9, PT=5, ST=4, MM=2, BC=2, STG=2, RC=2, QY=14, GPB=1, TRIM=1, RAF=0)


def build_nc():
    nc = bacc.Bacc("TRN2", target_bir_lowering=False, debug=False, num_devices=NCORES)
    xT = nc.declare_dram_parameter("xT", [D, T], F32R, isOutput=False)
    wq = nc.declare_dram_parameter("wq", [D, 512], F32R, isOutput=False)
    wk = nc.declare_dram_parameter("wk", [D, 512], F32R, isOutput=False)
    wv = nc.declare_dram_parameter("wv", [D, 512], F32R, isOutput=False)
    wo = nc.declare_dram_parameter("wo", [512, D], F32R, isOutput=False)
    msk = nc.declare_dram_parameter("msk", [4, 128, TB], F32, isOutput=False)
    ones = nc.declare_dram_parameter("ones", [128, 64], F32R, isOutput=False)
    out = nc.declare_dram_parameter("out", [T, D], F32, isOutput=True)

    with TileContext(nc) as tc:
        with (
            tc.tile_pool(name="sb", bufs=1) as sb,
            tc.tile_pool(name="ps", bufs=1, space="PSUM") as ps,
        ):
            # ---------- constants + weights (DMA order: wq/xt0 interleaved
            # first so phase A's first matmuls start early) ----------
            ones_sb = sb.tile([128, 64], F32R, name="ones", tag="ones", bufs=1)
            nc.sync.dma_start(out=ones_sb, in_=ones[:, :])
            wq_sb, wk_sb, wv_sb = [], [], []
            xt0 = []
            for kd in range(NKD):
                tq = sb.tile([128, 512], F32R, name=f"wq{kd}", tag="w", bufs=24)
                nc.sync.dma_start(out=tq, in_=wq[kd * 128:(kd + 1) * 128, :])
                wq_sb.append(tq)
                t = sb.tile([128, TB], F32R, name=f"xt0_{kd}", tag="xp", bufs=CFG["XP"])
                nc.sync.dma_start(out=t, in_=xT[kd * 128:(kd + 1) * 128, 0:TB])
                xt0.append(t)
            for kd in range(NKD):
                tk = sb.tile([128, 512], F32R, name=f"wk{kd}", tag="w", bufs=24)
                nc.sync.dma_start(out=tk, in_=wk[kd * 128:(kd + 1) * 128, :])
                wk_sb.append(tk)
            for kd in range(NKD):
                tv = sb.tile([128, 512], F32R, name=f"wv{kd}", tag="w", bufs=24)
                nc.sync.dma_start(out=tv, in_=wv[kd * 128:(kd + 1) * 128, :])
                wv_sb.append(tv)
            msk_sb = []
            for m in range(4):
                mt = sb.tile([128, TB], F32, name=f"msk{m}", tag=f"msk{m}", bufs=1)
                nc.sync.dma_start(out=mt, in_=msk[m])
                msk_sb.append(mt)
            wo_sb = []
            for r in range(4):
                to = sb.tile([128, D], F32R, name=f"wo{r}", tag="wo", bufs=4)
                nc.sync.dma_start(out=to, in_=wo[r * 128:(r + 1) * 128, :])
                wo_sb.append(to)

            qT_sb = {}
            kT_sb = [sb.tile([128, T], F32R, name=f"kT{hp}", tag="kt", bufs=4)
                     for hp in range(NHP)]
            v_sb = [sb.tile([128, 8 * 65], F32R, name=f"v{ti}", tag="v", bufs=16)
                    for ti in range(NT)]
            yT_sb = {}

            def qkv_block(tb, xt):
                # q/k transposed projections: psum [w-cols 128, t 512]
                for sec, w_tiles in enumerate((wq_sb, wk_sb)):
                    for mc in range(4):
                        pacc = ps.tile([128, TB], F32, name=f"pqk{tb}_{sec}_{mc}",
                                       tag="mm", bufs=CFG["MM"])
                        for kd in range(NKD):
                            nc.tensor.matmul(
                                pacc, lhsT=w_tiles[kd][:, mc * 128:(mc + 1) * 128],
                                rhs=xt[kd], start=(kd == 0), stop=(kd == NKD - 1))
                        if sec == 0:
                            qt = sb.tile([128, TB], F32R, name=f"qT{mc}_{tb}",
                                         tag="qy", bufs=CFG["QY"])
                            qT_sb[(mc, tb)] = qt
                            nc.vector.tensor_copy(qt, pacc)
                        else:
                            nc.vector.tensor_copy(
                                kT_sb[mc][:, tb * TB:(tb + 1) * TB], pacc)
                # v natural projection: psum [t 128, 512 cols]
                for tt in range(4):
                    ti = tb * 4 + tt
                    pv = ps.tile([128, 512], F32, name=f"pv{ti}", tag="mm",
                                 bufs=CFG["MM"])
                    for kd in range(NKD):
                        nc.tensor.matmul(
                            pv, lhsT=xt[kd][:, tt * 128:(tt + 1) * 128],
                            rhs=wv_sb[kd], start=(kd == 0), stop=(kd == NKD - 1))
                    v3 = v_sb[ti].rearrange("p (h c) -> p h c", h=8)
                    nc.vector.tensor_copy(
                        v3[:, :, 0:64], pv.rearrange("p (h c) -> p h c", h=8))
                    nc.vector.tensor_copy(
                        v3[:, :, 64:65],
                        ones_sb[:, 0:8].rearrange("p (h c) -> p h c", h=8))

            def attn_block(hp, qb):
                jmax = 4 * qb + 4
                ypair = [ps.tile([65, TB], F32, name=f"y{hp}_{qb}_{i}",
                                 tag="y", bufs=2) for i in range(2)]
                for j in range(jmax):
                    stp = [ps.tile([128, TB], F32, name=f"st{hp}_{qb}_{j}_{i}",
                                   tag="st", bufs=CFG["ST"]) for i in range(2)]
                    for i in range(2):
                        nc.tensor.matmul(
                            stp[i],
                            lhsT=kT_sb[hp][i * 64:(i + 1) * 64, j * 128:(j + 1) * 128],
                            rhs=qT_sb[(hp, qb)][i * 64:(i + 1) * 64, :],
                            start=True, stop=True, tile_position=(i * 64, 0))
                    m = j - 4 * qb
                    lo = max(m, 0) * 128 if CFG["TRIM"] else 0
                    for i in range(2):
                        if m >= 0:
                            nc.vector.tensor_tensor(
                                out=stp[i][:, lo:], in0=stp[i][:, lo:],
                                in1=msk_sb[m][:, lo:], op=Alu.add)
                        pt = sb.tile([128, TB], F32R, name=f"pt{hp}_{qb}_{j}_{i}",
                                     tag="pt", bufs=CFG["PT"])
                        nc.scalar.activation(pt[:, lo:], stp[i][:, lo:], Exp)
                        nc.tensor.matmul(
                            ypair[i][:, lo:],
                            lhsT=v_sb[j][:, (2 * hp + i) * 65:(2 * hp + i + 1) * 65],
                            rhs=pt[:, lo:], start=(j == 0), stop=(j == jmax - 1))
                # normalization: yT = y / sums
                for i in range(2):
                    bc = sb.tile([64, TB], F32, name=f"bc{hp}_{qb}_{i}",
                                 tag="bc", bufs=CFG["BC"])
                    if CFG["GPB"]:
                        rc = sb.tile([1, TB], F32, name=f"rc{hp}_{qb}_{i}",
                                     tag="rc", bufs=CFG["RC"])
                        if CFG.get("RAF", 1):
                            nc.vector.reciprocal_approx_fast(rc, ypair[i][64:65, :])
                        else:
                            nc.vector.reciprocal(rc, ypair[i][64:65, :])
                        nc.gpsimd.partition_broadcast(bc, rc)
                    else:
                        rc = sb.tile([1, TB], F32R, name=f"rc{hp}_{qb}_{i}",
                                     tag="rc", bufs=CFG["RC"])
                        with nc.allow_low_precision(reason="softmax denom"):
                            nc.vector.reciprocal(rc, ypair[i][64:65, :])
                        bps = ps.tile([64, TB], F32, name=f"b{hp}_{qb}_{i}",
                                      tag="bx", bufs=1)
                        nc.tensor.matmul(bps, lhsT=ones_sb[0:1, :],
                                         rhs=rc, start=True, stop=True)
                        nc.vector.tensor_copy(bc, bps)
                    yt = sb.tile([128, TB], F32R, name=f"yt{hp}_{qb}",
                                 tag="qy", bufs=CFG["QY"]) if i == 0 else yT_sb[(hp, qb)]
                    yT_sb[(hp, qb)] = yt
                    nc.vector.tensor_tensor(
                        out=yt[i * 64:(i + 1) * 64, :],
                        in0=ypair[i][0:64, :], in1=bc, op=Alu.mult)

            def outproj(qb):
                for tt in range(4):
                    ti = qb * 4 + tt
                    po = [ps.tile([128, 512], F32, name=f"po{ti}_{e}", tag="st",
                                  bufs=CFG["ST"]) for e in range(2)]
                    for r in range(4):
                        lhsT = yT_sb[(r, qb)][:, tt * 128:(tt + 1) * 128]
                        for e in range(2):
                            nc.tensor.matmul(po[e], lhsT=lhsT,
                                             rhs=wo_sb[r][:, e * 512:(e + 1) * 512],
                                             start=(r == 0), stop=(r == 3))
                    for e in range(2):
                        stg = sb.tile([128, 512], F32, name=f"stg{ti}_{e}",
                                      tag="stg", bufs=CFG["STG"])
                        nc.vector.tensor_copy(stg, po[e])
                        nc.sync.dma_start(
                            out=out[ti * 128:(ti + 1) * 128, e * 512:(e + 1) * 512],
                            in_=stg)

            for tb in range(NTB):
                if tb == 0:
                    xt = xt0
                else:
                    xt = []
                    for kd in range(NKD):
                        t = sb.tile([128, TB], F32R, name=f"xt{tb}_{kd}", tag="xp",
                                    bufs=CFG["XP"])
                        nc.sync.dma_start(
                            out=t, in_=xT[kd * 128:(kd + 1) * 128, tb * TB:(tb + 1) * TB])
                        xt.append(t)
                qkv_block(tb, xt)
                if CFG["FUSE"]:
                    for hp in range(NHP):
                        attn_block(hp, tb)
                    outproj(tb)
            if not CFG["FUSE"]:
                for qb in range(NTB):
                    for hp in range(NHP):
                        attn_block(hp, qb)
                    outproj(qb)
    nc.compile()
    return nc


def make_in_maps(x, w_qkv, w_out):
    x = np.asarray(x, np.float32)
    w_qkv = np.asarray(w_qkv, np.float32)
    w_out = np.asarray(w_out, np.float32)
    # causal mask patterns for the 4 diagonal j-tile offsets (ST layout: j on
    # partitions, q on free dim): allowed iff m*128 + p <= f
    m_idx = np.arange(4)[:, None, None] * 128 + np.arange(128)[None, :, None]
    f_idx = np.arange(TB)[None, None, :]
    msk = np.where(m_idx <= f_idx, 0.0, NEG).astype(np.float32)
    ones = np.ones((128, 64), np.float32)
    in_maps = []
    for c in range(NCORES):
        b, hg = divmod(c, 2)
        cs = slice(hg * 512, (hg + 1) * 512)
        in_maps.append({
            "xT": np.ascontiguousarray(x[b].T),
            "wq": np.ascontiguousarray(w_qkv[:, 0:D][:, cs] * 0.125),
            "wk": np.ascontiguousarray(w_qkv[:, D:2 * D][:, cs]),
            "wv": np.ascontiguousarray(w_qkv[:, 2 * D:3 * D][:, cs]),
            "wo": np.ascontiguousarray(w_out[cs, :]),
            "msk": msk,
            "ones": ones,
        })
    return in_maps


_NC_CACHE = []


def kernel(x, w_qkv, w_out):
    if not _NC_CACHE:
        _NC_CACHE.append(build_nc())
    nc = _NC_CACHE[0]
    in_maps = make_in_maps(x, w_qkv, w_out)
    res = None
    for attempt in range(3):
        try:
            res = run_bass_kernel_spmd(nc, in_maps, list(range(NCORES))).results
            break
        except Exception:
            # transient NRT device errors recover on retry
            if attempt == 2:
                raise
    out = np.empty((B, T, D), np.float32)
    for b in range(B):
        out[b] = res[2 * b]["out"] + res[2 * b + 1]["out"]
    return out


if __name__ == "__main__":
    rng = np.random.default_rng(0)
    x = rng.standard_normal((B, T, D)).astype(np.float32)
    w_qkv = (rng.standard_normal((D, 3 * D)) / np.sqrt(D)).astype(np.float32)
    w_out = (rng.standard_normal((D, D)) / np.sqrt(D)).astype(np.float32)
    y = kernel(x, w_qkv, w_out)
    print("ran ok", y.shape, y.dtype)
